# revision 1
# baseline (speedup 1.0000x reference)
"""Trainium2 Bass kernel for a Mistral-style cross-attention transformer block.

Sharding (8 NeuronCores, tensor-parallel, zero on-device collectives):
  Launch 1 (attention): cores grouped by batch (4 cores/batch); each core
    computes 8 q-heads / 2 kv-heads of cross-attention for its batch and a
    partial O-projection (contraction over its context dims). Host sums the
    4 partials per batch and adds the residual.
  Launch 2 (MLP): classic Megatron split of the SwiGLU intermediate dim
    (2048 per core); each core emits a partial down-projection. Host sums
    partials and adds the residual.

All matmuls run as float32r (TF32-like, full PE rate at >=256 moving rows,
~1e-4 relative error); softmax/LayerNorm arithmetic is float32.
"""
import numpy as np
import os

import concourse.mybir as mybir
import concourse.tile as tile
from concourse import bacc
from concourse.bass_utils import run_bass_kernel_spmd
from concourse.masks import make_identity

B, QL, KVL, D = 2, 1024, 2048, 4096
NH, NKV, HD = 32, 8, 128
INNER = 4 * D
EPS = 1e-5
THETA = 10000.0
NCORES = 8
P = 128
F32 = mybir.dt.float32
F32R = mybir.dt.float32r
AX = mybir.AxisListType.X
ALU = mybir.AluOpType
ACTF = mybir.ActivationFunctionType

H_LOC = NH // (NCORES // B)      # 8 q heads per core
KV_LOC = NKV // (NCORES // B)    # 2 kv heads per core
J_LOC = INNER // NCORES          # 2048 intermediate dims per core
DK = D // P                      # 32 k-tiles over hidden dim


def _ln_normalize(nc, stat, scratch, x_t, eps_ap, width):
    """rstd/neg_mu_rstd [128,1] from [128,width] f32 tile: h = x*rstd + nmur."""
    s1 = stat.tile([P, 1], F32, name="s1")
    s2 = stat.tile([P, 1], F32, name="s2")
    sq = scratch.tile([P, width], F32, name="sq")
    nc.vector.reduce_sum(s1[:], x_t[:], axis=AX)
    nc.scalar.square(sq[:], x_t[:])
    nc.vector.reduce_sum(s2[:], sq[:], axis=AX)
    mu = stat.tile([P, 1], F32, name="mu")
    m2 = stat.tile([P, 1], F32, name="m2")
    nc.vector.tensor_scalar_mul(mu[:], s1[:], 1.0 / width)
    nc.vector.tensor_scalar_mul(m2[:], s2[:], 1.0 / width)
    musq = stat.tile([P, 1], F32, name="musq")
    var = stat.tile([P, 1], F32, name="var")
    nc.vector.tensor_tensor(out=musq[:], in0=mu[:], in1=mu[:], op=ALU.mult)
    nc.vector.tensor_tensor(out=var[:], in0=m2[:], in1=musq[:], op=ALU.subtract)
    std = stat.tile([P, 1], F32, name="std")
    nc.scalar.activation(std[:], var[:], ACTF.Sqrt, bias=eps_ap)
    rstd = stat.tile([P, 1], F32, name="rstd")
    nc.vector.reciprocal(rstd[:], std[:])
    nmur = stat.tile([P, 1], F32, name="nmur")
    nc.vector.tensor_scalar(
        out=nmur[:], in0=mu[:], scalar1=rstd[:], scalar2=-1.0,
        op0=ALU.mult, op1=ALU.mult,
    )
    return rstd, nmur


def _rope_from_psum(nc, scratch, psrc, cos_ap, sin_ap, out_ap, width):
    """out = psrc*cos + shiftswap(psrc)*sin ; psrc is a PSUM [128,width] AP.

    Cross-partition moves are done with single-input copies (PSUM->SB), since
    two-SBUF-input ops require equal base partitions on trn2.
    """
    half = HD // 2
    rot = scratch.tile([P, width], F32, name="rope_rot")
    nc.vector.tensor_copy(rot[0:half, :], psrc[half:P, :])
    nc.vector.tensor_copy(rot[half:P, :], psrc[0:half, :])
    t2 = scratch.tile([P, width], F32, name="rope_t2")
    nc.vector.tensor_tensor(out=t2[:], in0=psrc, in1=cos_ap[:], op=ALU.mult)
    t1 = scratch.tile([P, width], F32, name="rope_t1")
    nc.vector.tensor_tensor(out=t1[:], in0=rot[:], in1=sin_ap[:], op=ALU.mult)
    nc.vector.tensor_tensor(out=out_ap, in0=t2[:], in1=t1[:], op=ALU.add)


def build_l1():
    nc = bacc.Bacc("TRN2", target_bir_lowering=False, debug=False)

    x = nc.dram_tensor("x", (QL, D), F32, kind="ExternalInput")
    encT = nc.dram_tensor("encT", (D, KVL), F32R, kind="ExternalInput")
    qwT = nc.dram_tensor("qwT", (D, H_LOC * HD), F32R, kind="ExternalInput")
    kwT = nc.dram_tensor("kwT", (D, KV_LOC * HD), F32R, kind="ExternalInput")
    vwT = nc.dram_tensor("vwT", (D, KV_LOC * HD), F32R, kind="ExternalInput")
    owT = nc.dram_tensor("owT", (H_LOC * HD, D), F32R, kind="ExternalInput")
    ln1w = nc.dram_tensor("ln1w", (D, 1), F32, kind="ExternalInput")
    ln1b = nc.dram_tensor("ln1b", (D, 1), F32, kind="ExternalInput")
    cosq = nc.dram_tensor("cosq", (HD, QL), F32, kind="ExternalInput")
    sinq = nc.dram_tensor("sinq", (HD, QL), F32, kind="ExternalInput")
    cosk = nc.dram_tensor("cosk", (HD, KVL), F32, kind="ExternalInput")
    sink = nc.dram_tensor("sink", (HD, KVL), F32, kind="ExternalInput")
    mask = nc.dram_tensor("mask", (1, KVL), F32, kind="ExternalInput")
    attn_part = nc.dram_tensor("attn_part", (QL, D), F32, kind="ExternalOutput")

    TQ = QL // P   # 8 query tiles
    TK = KVL // P  # 16 key tiles

    with tile.TileContext(nc) as tc:
        with (
            tc.tile_pool(name="const", bufs=1) as const,
            tc.tile_pool(name="stat", bufs=4) as stat,
            tc.tile_pool(name="res", bufs=1) as res,
            tc.tile_pool(name="dram", bufs=1, space="DRAM") as dram,
        ):
            id32 = const.tile([P, P], F32, name="id32")
            make_identity(nc, id32[:])
            ident = const.tile([P, P], F32R, name="ident")
            nc.vector.tensor_copy(ident[:], id32[:])
            eps_t = const.tile([P, 1], F32, name="eps_t")
            nc.vector.memset(eps_t[:], EPS)
            eps_ap = eps_t[:]

            ln1w_sb = const.tile([P, DK], F32, name="ln1w")
            ln1b_sb = const.tile([P, DK], F32, name="ln1b")
            nc.sync.dma_start(out=ln1w_sb[:], in_=ln1w.rearrange("(ko p) o -> p (ko o)", p=P))
            nc.sync.dma_start(out=ln1b_sb[:], in_=ln1b.rearrange("(ko p) o -> p (ko o)", p=P))

            cosq_sb = const.tile([HD, QL], F32, name="cosq")
            sinq_sb = const.tile([HD, QL], F32, name="sinq")
            cosk_sb = const.tile([HD, KVL], F32, name="cosk")
            sink_sb = const.tile([HD, KVL], F32, name="sink")
            for t, src in ((cosq_sb, cosq), (sinq_sb, sinq), (cosk_sb, cosk), (sink_sb, sink)):
                nc.sync.dma_start(out=t[:], in_=src[:, :])

            # mask -> exp(mask != 0) broadcast to [128, KVL] via K=1 matmul
            mrow = const.tile([1, KVL], F32, name="mrow")
            nc.sync.dma_start(out=mrow[:], in_=mask[:, :])
            mneq = const.tile([1, KVL], F32R, name="mneq")
            nc.vector.tensor_scalar(out=mneq[:], in0=mrow[:], scalar1=0.0, scalar2=None,
                                    op0=ALU.not_equal)
            ones32 = const.tile([1, P], F32, name="ones32")
            nc.vector.memset(ones32[:], 1.0)
            ones_col = const.tile([1, P], F32R, name="ones_col")
            nc.vector.tensor_copy(ones_col[:], ones32[:])

            kT = res.tile([P, KV_LOC, KVL], F32R, name="kT")
            v_nat = res.tile([P, KV_LOC, TK, HD], F32R, name="v_nat")
            hT_d = dram.tile([D, QL], F32R, name="hT_d")
            qT_d = dram.tile([H_LOC * HD, QL], F32R, name="qT_d")
            ctx_d = dram.tile([H_LOC * HD, QL], F32R, name="ctx_d")

            # ======== Phases A (LN1+hT) and C (K/V) — independent, overlap ========
            with (
                tc.tile_pool(name="ac_s", bufs=1) as ab1,
                tc.tile_pool(name="ac_s2", bufs=2) as ab2,
                tc.tile_pool(name="ac_s3", bufs=3) as ab3,
                tc.tile_pool(name="ac_tps", bufs=2, space="PSUM") as acps,
                tc.tile_pool(name="ac_kv", bufs=1, space="PSUM") as kvps,
                tc.tile_pool(name="ac_vtr", bufs=1, space="PSUM") as vtps,
            ):
                # ---- Phase A: LN1 + transpose -> hT (DRAM scratch) ----
                for tt in range(TQ if "A" in os.environ.get("L1_PHASES", "ACBDE") else 0):
                    x_t = ab1.tile([P, D], F32, name="x_t")
                    nc.sync.dma_start(out=x_t[:], in_=x[tt * P:(tt + 1) * P, :])
                    rstd, nmur = _ln_normalize(nc, stat, ab1, x_t, eps_ap, D)
                    tmp = ab1.tile([P, D], F32R, name="tmp")
                    nc.vector.tensor_scalar(
                        out=tmp[:], in0=x_t[:], scalar1=rstd[:], scalar2=nmur[:],
                        op0=ALU.mult, op1=ALU.add,
                    )
                    for k in range(DK):
                        ps = acps.tile([P, 512], F32R, name="trp")
                        nc.tensor.transpose(ps[:, :P], tmp[:, k * P:(k + 1) * P], ident[:])
                        hT_sb = ab3.tile([P, P], F32R, name="hT_sb")
                        nc.vector.tensor_scalar(
                            out=hT_sb[:], in0=ps[:, :P],
                            scalar1=ln1w_sb[:, k:k + 1], scalar2=ln1b_sb[:, k:k + 1],
                            op0=ALU.mult, op1=ALU.add,
                        )
                        nc.sync.dma_start(
                            out=hT_d[k * P:(k + 1) * P, tt * P:(tt + 1) * P], in_=hT_sb[:])

                # ---- Phase C: K/V projections, tk-chunks of 512 ----
                for chunk in range(4 if "C" in os.environ.get("L1_PHASES", "ACBDE") else 0):
                    ck = slice(chunk * 512, (chunk + 1) * 512)
                    pk = [kvps.tile([P, 512], F32, name=f"pk{i}") for i in range(KV_LOC)]
                    pv = [kvps.tile([P, 512], F32, name=f"pv{i}") for i in range(KV_LOC)]
                    for k in range(DK):
                        encT_k = ab3.tile([P, 512], F32R, name="encT_k")
                        nc.sync.dma_start(out=encT_k[:], in_=encT[k * P:(k + 1) * P, ck])
                        kwT_k = ab3.tile([P, KV_LOC * HD], F32R, name="kwT_k")
                        vwT_k = ab3.tile([P, KV_LOC * HD], F32R, name="vwT_k")
                        nc.sync.dma_start(out=kwT_k[:], in_=kwT[k * P:(k + 1) * P, :])
                        nc.sync.dma_start(out=vwT_k[:], in_=vwT[k * P:(k + 1) * P, :])
                        for kv in range(KV_LOC):
                            nc.tensor.matmul(pk[kv][:], kwT_k[:, kv * P:(kv + 1) * P],
                                             encT_k[:], start=(k == 0), stop=(k == DK - 1))
                            nc.tensor.matmul(pv[kv][:], vwT_k[:, kv * P:(kv + 1) * P],
                                             encT_k[:], start=(k == 0), stop=(k == DK - 1))
                    for kv in range(KV_LOC):
                        _rope_from_psum(nc, ab2, pk[kv][:], cosk_sb[:, ck], sink_sb[:, ck],
                                        kT[:, kv, ck], 512)
                        vT_sb = ab2.tile([P, 512], F32R, name="vT_sb")
                        nc.vector.tensor_copy(vT_sb[:], pv[kv][:])
                        for tkl in range(4):
                            tk = chunk * 4 + tkl
                            psv = vtps.tile([P, P], F32R, name="vtr")
                            nc.tensor.transpose(psv[:], vT_sb[:, tkl * P:(tkl + 1) * P],
                                                ident[:])
                            nc.vector.tensor_copy(v_nat[:, kv, tk, :], psv[:])

            # ======== Phase B: Q projection + RoPE -> qT_d ========
            with (
                tc.tile_pool(name="b_s2", bufs=2) as bb2,
                tc.tile_pool(name="b_s3", bufs=3) as bb3,
                tc.tile_pool(name="b_ps", bufs=1, space="PSUM") as bps,
            ):
                for quad in range(2 if "B" in os.environ.get("L1_PHASES", "ACBDE") else 0):
                    pq = [bps.tile([P, QL], F32, name=f"pq{i}") for i in range(4)]
                    for k in range(DK):
                        hT_k = bb3.tile([P, QL], F32R, name="hT_k")
                        nc.sync.dma_start(out=hT_k[:], in_=hT_d[k * P:(k + 1) * P, :])
                        qwT_k = bb3.tile([P, 512], F32R, name="qwT_k")
                        nc.sync.dma_start(
                            out=qwT_k[:],
                            in_=qwT[k * P:(k + 1) * P, quad * 512:(quad + 1) * 512])
                        for hq in range(4):
                            for c2 in range(2):
                                nc.tensor.matmul(
                                    pq[hq][:, c2 * 512:(c2 + 1) * 512],
                                    qwT_k[:, hq * P:(hq + 1) * P],
                                    hT_k[:, c2 * 512:(c2 + 1) * 512],
                                    start=(k == 0), stop=(k == DK - 1),
                                )
                    for hq in range(4):
                        h = quad * 4 + hq
                        qrope = bb2.tile([P, QL], F32R, name="qrope")
                        _rope_from_psum(nc, bb2, pq[hq][:], cosq_sb[:], sinq_sb[:],
                                        qrope[:], QL)
                        nc.sync.dma_start(out=qT_d[h * P:(h + 1) * P, :], in_=qrope[:])

            # ======== Phase D: attention ========
            with (
                tc.tile_pool(name="d_p", bufs=1) as dp,
                tc.tile_pool(name="d_s2", bufs=2) as ds2,
                tc.tile_pool(name="d_ps", bufs=2, space="PSUM") as dps,
                tc.tile_pool(name="d_ps4", bufs=2, space="PSUM") as dps4,
            ):
                for h in range(H_LOC if "D" in os.environ.get("L1_PHASES", "ACBDE") else 0):
                    kv = h // (H_LOC // KV_LOC)
                    qT_h = ds2.tile([P, QL], F32R, name="qT_h")
                    nc.sync.dma_start(out=qT_h[:], in_=qT_d[h * P:(h + 1) * P, :])
                    for tp in range(TQ // 2):
                        p_sb = dp.tile([P, 2, KVL], F32, name="p_sb")
                        pn_sb = dp.tile([P, 2, KVL], F32R, name="pn_sb")
                        rs = stat.tile([P, 2], F32, name="rs")
                        for ti in range(2):
                            tq = tp * 2 + ti
                            sums = stat.tile([P, 4], F32, name="sums")
                            for tkc in range(4):
                                ps_s = dps4.tile([P, 512], F32, name="ps_s")
                                nc.tensor.matmul(
                                    ps_s[:], qT_h[:, tq * P:(tq + 1) * P],
                                    kT[:, kv, tkc * 512:(tkc + 1) * 512],
                                    start=True, stop=False,
                                )
                                nc.tensor.matmul(
                                    ps_s[:], ones_col[:],
                                    mneq[:, tkc * 512:(tkc + 1) * 512],
                                    start=False, stop=True,
                                )
                                nc.scalar.activation(p_sb[:, ti, tkc * 512:(tkc + 1) * 512],
                                                     ps_s[:], ACTF.Exp,
                                                     accum_out=sums[:, tkc:tkc + 1])
                            ssum = stat.tile([P, 1], F32, name="ssum")
                            nc.vector.reduce_sum(ssum[:], sums[:], axis=AX)
                            nc.vector.reciprocal(rs[:, ti:ti + 1], ssum[:])
                            nc.vector.tensor_scalar_mul(pn_sb[:, ti, :], p_sb[:, ti, :],
                                                        rs[:, ti:ti + 1])
                        pt_sb = dp.tile([P, TK, 2 * P], F32R, name="pt_sb")
                        for tk in range(TK):
                            for ti in range(2):
                                pstr = dps4.tile([P, P], F32R, name="pttr")
                                nc.tensor.transpose(pstr[:],
                                                    pn_sb[:, ti, tk * P:(tk + 1) * P],
                                                    ident[:])
                                nc.vector.tensor_copy(pt_sb[:, tk, ti * P:(ti + 1) * P],
                                                      pstr[:])
                        ps_c = dps.tile([P, 2 * P], F32, name="ps_c")
                        for tk in range(TK):
                            nc.tensor.matmul(ps_c[:], v_nat[:, kv, tk, :], pt_sb[:, tk, :],
                                             start=(tk == 0), stop=(tk == TK - 1))
                        ctx_sb = ds2.tile([P, 2 * P], F32R, name="ctx_sb")
                        nc.vector.tensor_copy(ctx_sb[:], ps_c[:])
                        nc.sync.dma_start(
                            out=ctx_d[h * P:(h + 1) * P, tp * 2 * P:(tp + 1) * 2 * P],
                            in_=ctx_sb[:])

            # ======== Phase E: partial O projection ========
            with (
                tc.tile_pool(name="e_w", bufs=2) as ew,
                tc.tile_pool(name="e_s", bufs=3) as es,
                tc.tile_pool(name="e_ps", bufs=4, space="PSUM") as eps_pool,
            ):
                for do in range(D // 512 if "E" in os.environ.get("L1_PHASES", "ACBDE") else 0):
                    owT_do = ew.tile([P, H_LOC, 512], F32R, name="owT_do")
                    for k in range(H_LOC):
                        nc.sync.dma_start(
                            out=owT_do[:, k, :],
                            in_=owT[k * P:(k + 1) * P, do * 512:(do + 1) * 512])
                    for tq in range(TQ):
                        ctx_t = es.tile([P, H_LOC, P], F32R, name="ctx_t")
                        for k in range(H_LOC):
                            nc.sync.dma_start(
                                out=ctx_t[:, k, :],
                                in_=ctx_d[k * P:(k + 1) * P, tq * P:(tq + 1) * P])
                        ps_o = eps_pool.tile([P, 512], F32, name="ps_o")
                        for k in range(H_LOC):
                            nc.tensor.matmul(ps_o[:], ctx_t[:, k, :], owT_do[:, k, :],
                                             start=(k == 0), stop=(k == H_LOC - 1))
                        o_sb = es.tile([P, 512], F32, name="o_sb")
                        nc.vector.tensor_copy(o_sb[:], ps_o[:])
                        nc.sync.dma_start(
                            out=attn_part[tq * P:(tq + 1) * P, do * 512:(do + 1) * 512],
                            in_=o_sb[:])
    nc.compile()
    return nc


def build_l2():
    nc = bacc.Bacc("TRN2", target_bir_lowering=False, debug=False)

    h1 = nc.dram_tensor("h1", (B * QL, D), F32, kind="ExternalInput")
    gwT = nc.dram_tensor("gwT", (D, J_LOC), F32R, kind="ExternalInput")
    uwT = nc.dram_tensor("uwT", (D, J_LOC), F32R, kind="ExternalInput")
    dwT = nc.dram_tensor("dwT", (J_LOC, D), F32R, kind="ExternalInput")
    ln2w = nc.dram_tensor("ln2w", (D, 1), F32, kind="ExternalInput")
    ln2b = nc.dram_tensor("ln2b", (D, 1), F32, kind="ExternalInput")
    ff_part = nc.dram_tensor("ff_part", (B * QL, D), F32, kind="ExternalOutput")

    T_BLK = 512
    NBLK = B * QL // T_BLK           # 4
    TPB = T_BLK // P                 # 4 token tiles per block
    JK = J_LOC // P                  # 16 j-tiles

    with tile.TileContext(nc) as tc:
        with (
            tc.tile_pool(name="const", bufs=1) as const,
            tc.tile_pool(name="stat", bufs=4) as stat,
            tc.tile_pool(name="dram", bufs=1, space="DRAM") as dram,
        ):
            id32 = const.tile([P, P], F32, name="id32")
            make_identity(nc, id32[:])
            ident = const.tile([P, P], F32R, name="ident")
            nc.vector.tensor_copy(ident[:], id32[:])
            eps_t = const.tile([P, 1], F32, name="eps_t")
            nc.vector.memset(eps_t[:], EPS)
            eps_ap = eps_t[:]
            ln2w_sb = const.tile([P, DK], F32, name="ln2w")
            ln2b_sb = const.tile([P, DK], F32, name="ln2b")
            nc.sync.dma_start(out=ln2w_sb[:], in_=ln2w.rearrange("(ko p) o -> p (ko o)", p=P))
            nc.sync.dma_start(out=ln2b_sb[:], in_=ln2b.rearrange("(ko p) o -> p (ko o)", p=P))

            ff_d = dram.tile([J_LOC, B * QL], F32R, name="ff_d")

            with (
                tc.tile_pool(name="g_h2", bufs=1) as h2p,
                tc.tile_pool(name="g_s1", bufs=1) as gs1,
                tc.tile_pool(name="g_w", bufs=3) as gw,
                tc.tile_pool(name="g_e", bufs=2) as ge,
                tc.tile_pool(name="g_tps", bufs=4, space="PSUM") as gtps,
                tc.tile_pool(name="g_ps", bufs=1, space="PSUM") as gps,
            ):
                for tb in range(NBLK):
                    # ---- LN2 + transpose -> h2T block [d, T_BLK] ----
                    h2T = h2p.tile([P, DK, T_BLK], F32R, name="h2T")
                    for ti in range(TPB):
                        tt = tb * TPB + ti
                        x_t = gs1.tile([P, D], F32, name="x_t")
                        nc.sync.dma_start(out=x_t[:], in_=h1[tt * P:(tt + 1) * P, :])
                        rstd, nmur = _ln_normalize(nc, stat, gs1, x_t, eps_ap, D)
                        tmp = gs1.tile([P, D], F32R, name="tmp")
                        nc.vector.tensor_scalar(
                            out=tmp[:], in0=x_t[:], scalar1=rstd[:], scalar2=nmur[:],
                            op0=ALU.mult, op1=ALU.add,
                        )
                        for k in range(DK):
                            ps = gtps.tile([P, P], F32R, name="trp")
                            nc.tensor.transpose(ps[:], tmp[:, k * P:(k + 1) * P], ident[:])
                            nc.vector.tensor_scalar(
                                out=h2T[:, k, ti * P:(ti + 1) * P], in0=ps[:],
                                scalar1=ln2w_sb[:, k:k + 1], scalar2=ln2b_sb[:, k:k + 1],
                                op0=ALU.mult, op1=ALU.add,
                            )

                    # ---- gate/up + SwiGLU -> ff_d ----
                    for jp in range(JK // 2):
                        pg = [gps.tile([P, T_BLK], F32, name=f"pg{i}") for i in range(2)]
                        pu = [gps.tile([P, T_BLK], F32, name=f"pu{i}") for i in range(2)]
                        for k in range(DK):
                            gw_k = gw.tile([P, 2 * P], F32R, name="gw_k")
                            uw_k = gw.tile([P, 2 * P], F32R, name="uw_k")
                            nc.sync.dma_start(
                                out=gw_k[:],
                                in_=gwT[k * P:(k + 1) * P, jp * 2 * P:(jp + 1) * 2 * P])
                            nc.sync.dma_start(
                                out=uw_k[:],
                                in_=uwT[k * P:(k + 1) * P, jp * 2 * P:(jp + 1) * 2 * P])
                            for jj in range(2):
                                nc.tensor.matmul(pg[jj][:], gw_k[:, jj * P:(jj + 1) * P],
                                                 h2T[:, k, :], start=(k == 0),
                                                 stop=(k == DK - 1))
                                nc.tensor.matmul(pu[jj][:], uw_k[:, jj * P:(jj + 1) * P],
                                                 h2T[:, k, :], start=(k == 0),
                                                 stop=(k == DK - 1))
                        for jj in range(2):
                            g_sb = ge.tile([P, T_BLK], F32, name="g_sb")
                            nc.scalar.activation(g_sb[:], pg[jj][:], ACTF.Silu)
                            ff_sb = ge.tile([P, T_BLK], F32R, name="ff_sb")
                            nc.vector.tensor_tensor(out=ff_sb[:], in0=g_sb[:],
                                                    in1=pu[jj][:], op=ALU.mult)
                            j = jp * 2 + jj
                            nc.sync.dma_start(
                                out=ff_d[j * P:(j + 1) * P, tb * T_BLK:(tb + 1) * T_BLK],
                                in_=ff_sb[:])

            # ---- down projection (partial) ----
            TS = 1024
            with (
                tc.tile_pool(name="d_ff", bufs=1) as dff,
                tc.tile_pool(name="d_w", bufs=1) as dwp,
                tc.tile_pool(name="d_e", bufs=3) as de,
                tc.tile_pool(name="d_ps", bufs=2, space="PSUM") as dps,
            ):
                for ts2 in range(B * QL // TS):
                    ff_r = dff.tile([P, JK, TS], F32R, name="ff_r")
                    for jk in range(JK):
                        nc.sync.dma_start(
                            out=ff_r[:, jk, :],
                            in_=ff_d[jk * P:(jk + 1) * P, ts2 * TS:(ts2 + 1) * TS])
                    for do in range(D // 512):
                        dw_do = dwp.tile([P, JK, 512], F32R, name="dw_do")
                        for jk in range(JK):
                            nc.sync.dma_start(
                                out=dw_do[:, jk, :],
                                in_=dwT[jk * P:(jk + 1) * P, do * 512:(do + 1) * 512])
                        for tt2 in range(TS // P):
                            ps_d = dps.tile([P, 512], F32, name="ps_d")
                            for jk in range(JK):
                                nc.tensor.matmul(ps_d[:],
                                                 ff_r[:, jk, tt2 * P:(tt2 + 1) * P],
                                                 dw_do[:, jk, :], start=(jk == 0),
                                                 stop=(jk == JK - 1))
                            o_sb = de.tile([P, 512], F32, name="o_sb")
                            nc.vector.tensor_copy(o_sb[:], ps_d[:])
                            nc.sync.dma_start(
                                out=ff_part[ts2 * TS + tt2 * P: ts2 * TS + (tt2 + 1) * P,
                                            do * 512:(do + 1) * 512],
                                in_=o_sb[:])
    nc.compile()
    return nc


def _rope_tables(seq_len, scale):
    """cosT, sinT_signed [HD, seq_len] f32; sin rows 0:63 negated; both scaled."""
    exp = (np.arange(0, HD, 2).astype(np.float32) / np.float32(HD))
    inv_freq = (np.float32(1.0) / np.power(np.float32(THETA), exp)).astype(np.float32)
    t = np.arange(seq_len, dtype=np.float32)
    freqs = np.outer(t, inv_freq).astype(np.float32)        # [S, HD/2]
    emb = np.concatenate([freqs, freqs], axis=-1)           # [S, HD]
    cos = np.cos(emb).astype(np.float32).T * np.float32(scale)   # [HD, S]
    sin = np.sin(emb).astype(np.float32).T * np.float32(scale)
    sin_signed = sin.copy()
    sin_signed[: HD // 2] = -sin_signed[: HD // 2]
    return np.ascontiguousarray(cos), np.ascontiguousarray(sin_signed)


_CACHE = {}


def _get(name, builder):
    if name not in _CACHE:
        _CACHE[name] = builder()
    return _CACHE[name]


def kernel(hidden_states, encoder_hidden_states, encoder_attention_mask,
           ln1_w, ln1_b, q_w, k_w, v_w, o_w, ln2_w, ln2_b,
           gate_w, up_w, down_w):
    f32 = np.float32
    hs = np.asarray(hidden_states, f32)
    enc = np.asarray(encoder_hidden_states, f32)
    am = np.asarray(encoder_attention_mask, f32)
    C = np.ascontiguousarray

    cosq, sinq = _rope_tables(QL, 1.0 / np.sqrt(np.float32(HD)))
    cosk, sink = _rope_tables(KVL, 1.0)

    nc1 = _get("l1", build_l1)
    in_maps = []
    for c in range(NCORES):
        b = c // (NCORES // B)
        g = c % (NCORES // B)
        hsl = slice(g * H_LOC * HD, (g + 1) * H_LOC * HD)
        kvsl = slice(g * KV_LOC * HD, (g + 1) * KV_LOC * HD)
        in_maps.append({
            "x": C(hs[b]),
            "encT": C(enc[b].T),
            "qwT": C(np.asarray(q_w, f32)[hsl, :].T),
            "kwT": C(np.asarray(k_w, f32)[kvsl, :].T),
            "vwT": C(np.asarray(v_w, f32)[kvsl, :].T),
            "owT": C(np.asarray(o_w, f32)[:, hsl].T),
            "ln1w": C(np.asarray(ln1_w, f32).reshape(D, 1)),
            "ln1b": C(np.asarray(ln1_b, f32).reshape(D, 1)),
            "cosq": cosq, "sinq": sinq, "cosk": cosk, "sink": sink,
            "mask": C(am[b].reshape(1, KVL)),
        })
    res1 = run_bass_kernel_spmd(nc1, in_maps, core_ids=list(range(NCORES)))

    h1 = hs.copy()
    for c in range(NCORES):
        b = c // (NCORES // B)
        h1[b] += res1.results[c]["attn_part"]

    nc2 = _get("l2", build_l2)
    h1_flat = C(h1.reshape(B * QL, D))
    in_maps2 = []
    for c in range(NCORES):
        jsl = slice(c * J_LOC, (c + 1) * J_LOC)
        in_maps2.append({
            "h1": h1_flat,
            "gwT": C(np.asarray(gate_w, f32)[jsl, :].T),
            "uwT": C(np.asarray(up_w, f32)[jsl, :].T),
            "dwT": C(np.asarray(down_w, f32)[:, jsl].T),
            "ln2w": C(np.asarray(ln2_w, f32).reshape(D, 1)),
            "ln2b": C(np.asarray(ln2_b, f32).reshape(D, 1)),
        })
    res2 = run_bass_kernel_spmd(nc2, in_maps2, core_ids=list(range(NCORES)))

    out = h1_flat.copy()
    for c in range(NCORES):
        out += res2.results[c]["ff_part"]
    return out.reshape(B, QL, D)

